# revision 5
# baseline (speedup 1.0000x reference)
"""Trainium2 Bass kernel for nn_DiffAttention (GNN message passing).

Math (per edge i: src s_i -> dst n, dst sorted):
  d_i = (h_dst[n] - h_src[s_i]) @ W_fc.T ;  e_i = tanh(d_i @ w_attn)
  alpha = segment_softmax(e, dst);  out[n] = elu(sum_i alpha_i d_i)
Since e in [-1,1], softmax needs no max-subtraction:
  out[n] = elu(p_dst[n] - (sum_i w_i p_src[s_i]) / (sum_i w_i)),
  w_i = exp(tanh(q_dst[n] - q_src[s_i])), p = h @ W_fc.T, q = p @ w_attn.

Device strategy (8 cores, SPMD, edge-parallel by dst range):
  - node table [NPAD, 132] f32 rows [p_src|1|q_src|p_dst|0|q_dst], built
    sharded (1/8 per core) on PE, then AllGather.
  - per window (<=128 consecutive dst nodes, 16x128 edge slots):
    1 indirect row-gather of the window's node rows (p_dst + q_dst),
    16 indirect row-gathers of per-edge src rows,
    per tile: one-hot S01[edge,slot] = (iota == dst_local) on DVE,
    qd per edge = rowsum(S01 * qb) (qb = q_win broadcast via K=1 matmul),
    w = exp(tanh(qd - qs)) on ACT, rhs = w*[p_src|1] (ACT scale),
    PSUM[slot, 0:65] += S01.T @ rhs  accumulates [sum w*p | sum w].
  - epilogue per window: elu(p_dst - swp/sw) with zero-edge masking.
Host does only index prep (windowing, padding, int16/f32 casts) and
reassembles per-window 128-slot outputs into node order.
"""
import sys
sys.path.insert(0, "/opt/trn_rl_repo")
import numpy as np

N = 100000
D = 64
NC = 8
K = 16            # 128-edge tiles per window
WE = K * 128      # edge slots per window
WIN_NODES = 128
SHARD = 12544     # 98*128 rows built per core
NPAD = NC * SHARD # 100352
DUMMY = N         # zero row (h padded with zeros)
ROW = 132         # [p_src(0:64) | 1(64) | q_src(65) | p_dst(66:130) | 0(130) | q_dst(131)]
MAIN_REPEAT = 1   # test.py overrides for timing


# ---------------------------------------------------------------- host prep
def _partition_edges(dst):
    E = dst.shape[0]
    bounds, e_prev, n_prev = [], 0, 0
    for c in range(1, NC):
        s = (E * c) // NC
        while 0 < s < E and dst[s] == dst[s - 1]:
            s += 1
        node_split = int(dst[s]) if s < E else N
        bounds.append((e_prev, s, n_prev, node_split))
        e_prev, n_prev = s, node_split
    bounds.append((e_prev, E, n_prev, N))
    return bounds


def _build_windows(src, dst, e_lo, e_hi, n_lo, n_hi):
    counts = np.bincount(dst[e_lo:e_hi] - n_lo, minlength=n_hi - n_lo)
    assert counts.max() <= WE, f"node degree {counts.max()} > window capacity"
    starts = np.concatenate([[0], np.cumsum(counts)])
    srcs, dls, bases, nns = [], [], [], []
    n, n_total = 0, n_hi - n_lo
    while n < n_total:
        n_end = min(n + WIN_NODES, n_total)
        while starts[n_end] - starts[n] > WE:
            n_end -= 1
        ecnt = int(starts[n_end] - starts[n])
        elo = e_lo + int(starts[n])
        s = np.full(WE, DUMMY, np.int32)
        dl = np.full(WE, -1.0, np.float32)
        s[:ecnt] = src[elo:elo + ecnt]
        dl[:ecnt] = (dst[elo:elo + ecnt] - (n_lo + n)).astype(np.float32)
        srcs.append(s); dls.append(dl)
        bases.append(n_lo + n); nns.append(n_end - n)
        n = n_end
    return np.stack(srcs), np.stack(dls), np.array(bases), np.array(nns)


def _prep(src, dst):
    src = np.asarray(src, np.int64)
    dst = np.asarray(dst, np.int64)
    if np.any(np.diff(dst) < 0):  # tolerate unsorted edges
        order = np.argsort(dst, kind="stable")
        src, dst = src[order], dst[order]
    bounds = _partition_edges(dst)
    per_core = [_build_windows(src, dst, *b) for b in bounds]
    nW = max(p[0].shape[0] for p in per_core)
    cores = []
    for (s, dl, base, nn) in per_core:
        pad = nW - s.shape[0]
        if pad:
            s = np.concatenate([s, np.full((pad, WE), DUMMY, np.int32)])
            dl = np.concatenate([dl, np.full((pad, WE), -1.0, np.float32)])
            base = np.concatenate([base, np.full(pad, N, np.int64)])
            nn = np.concatenate([nn, np.zeros(pad, np.int64)])
        slot = base[:, None] + np.arange(WIN_NODES)[None, :]
        slot = np.where(np.arange(WIN_NODES)[None, :] < nn[:, None], slot, DUMMY)
        cores.append(dict(src=s, dst_local=dl, base=base, nn=nn,
                          slot_ids=slot.astype(np.int32)))
    return cores, nW


def _to_tiles(a):  # [nW, WE] -> [nW, 128, K]; [w,p,k] = edge k*128+p
    nW = a.shape[0]
    return np.ascontiguousarray(a.reshape(nW, K, 128).transpose(0, 2, 1))


# ---------------------------------------------------------------- device
def _build_program(nW, main_repeat):
    from concourse import bass, bacc, mybir, tile
    f32, i32, i16 = mybir.dt.float32, mybir.dt.int32, mybir.dt.int16

    nc = bacc.Bacc("TRN2", target_bir_lowering=False, debug=False,
                   num_devices=NC)
    hs_e = nc.dram_tensor("hs", [SHARD, D], f32, kind="ExternalInput")
    hd_e = nc.dram_tensor("hd", [SHARD, D], f32, kind="ExternalInput")
    wfc_e = nc.dram_tensor("wfc", [D, D], f32, kind="ExternalInput")
    wat_e = nc.dram_tensor("wat", [D, 1], f32, kind="ExternalInput")
    sidx_e = nc.dram_tensor("sidx", [nW, 128, K], i32, kind="ExternalInput")
    dloc_e = nc.dram_tensor("dloc", [nW, 128, K], f32, kind="ExternalInput")
    nid_e = nc.dram_tensor("nid", [nW, 128, 1], i32, kind="ExternalInput")
    res_e = nc.dram_tensor("res", [nW * 128, D], f32, kind="ExternalOutput")

    with tile.TileContext(nc) as tc:
        with tc.tile_pool(name="c", bufs=1) as cp, \
             tc.tile_pool(name="sb", bufs=3) as sp, \
             tc.tile_pool(name="dr", bufs=1, space="DRAM") as dp:
            pp = tc.alloc_tile_pool(name="psb", bufs=1, space="PSUM")
            # ---- constants (shipped in the NEFF, no gpsimd custom ops)
            ident_d = nc.inline_tensor(np.eye(128, dtype=np.float32),
                                       name="ident_c")
            iota_d = nc.inline_tensor(
                np.tile(np.arange(128, dtype=np.float32), (128, 1)),
                name="iota_c")
            ident = cp.tile([128, 128], f32)
            nc.sync.dma_start(out=ident[:], in_=ident_d[:])
            iotaf = cp.tile([128, 128], f32)
            nc.sync.dma_start(out=iotaf[:], in_=iota_d[:])
            ones_row = cp.tile([1, 128], f32)
            nc.vector.memset(ones_row[:], 1.0)
            ones_col = cp.tile([128, 1], f32)
            nc.vector.memset(ones_col[:], 1.0)

            # ---- weight prep: rhs_build [64, 66] = [W.T | 0 | W.T @ w_attn]
            wfc = cp.tile([D, D], f32)
            nc.sync.dma_start(out=wfc[:], in_=wfc_e[:])
            wat = cp.tile([D, 1], f32)
            nc.sync.dma_start(out=wat[:], in_=wat_e[:])
            wt_ps = pp.tile([D, D], f32, space="PSUM")
            nc.tensor.transpose(out=wt_ps[:], in_=wfc[:], identity=ident[:D, :D])
            v_ps = pp.tile([D, 1], f32, space="PSUM")
            nc.tensor.matmul(out=v_ps[:], lhsT=wfc[:], rhs=wat[:],
                             start=True, stop=True)
            rhsb = cp.tile([D, 66], f32)
            nc.vector.memset(rhsb[:], 0.0)
            nc.vector.tensor_copy(rhsb[:, 0:64], wt_ps[:])
            nc.vector.tensor_copy(rhsb[:, 65:66], v_ps[:])

            # ---- table build (this core's shard)
            tbl_sh = dp.tile([SHARD, ROW], f32)
            for j in range(SHARD // 128):
                r0 = j * 128
                hs = sp.tile([128, D], f32, tag="bh")
                nc.sync.dma_start(out=hs[:], in_=hs_e[r0:r0 + 128, :])
                hd = sp.tile([128, D], f32, tag="bh2")
                nc.sync.dma_start(out=hd[:], in_=hd_e[r0:r0 + 128, :])
                hsT_ps = pp.tile([D, 128], f32, space="PSUM", tag="bt")
                nc.tensor.transpose(out=hsT_ps[:], in_=hs[:], identity=ident[:])
                hsT = sp.tile([D, 128], f32, tag="bs")
                nc.vector.tensor_copy(hsT[:], hsT_ps[:])
                hdT_ps = pp.tile([D, 128], f32, space="PSUM", tag="bt2")
                nc.tensor.transpose(out=hdT_ps[:], in_=hd[:], identity=ident[:])
                hdT = sp.tile([D, 128], f32, tag="bs2")
                nc.vector.tensor_copy(hdT[:], hdT_ps[:])
                pb = pp.tile([128, ROW], f32, space="PSUM", tag="bp")
                nc.tensor.matmul(out=pb[:, 0:66], lhsT=hsT[:], rhs=rhsb[:],
                                 start=True, stop=True)
                nc.tensor.matmul(out=pb[:, 66:132], lhsT=hdT[:], rhs=rhsb[:],
                                 start=True, stop=True)
                tb = sp.tile([128, ROW], f32, tag="bo")
                nc.vector.tensor_copy(tb[:], pb[:])
                nc.vector.memset(tb[:, 64:65], 1.0)
                nc.sync.dma_start(out=tbl_sh[r0:r0 + 128, :], in_=tb[:])

            pp.release()
            pp2 = tc.alloc_tile_pool(name="psm", bufs=2, space="PSUM")

            # ---- all-gather the table
            table = dp.tile([NPAD, ROW], f32)
            nc.gpsimd.collective_compute(
                "AllGather", mybir.AluOpType.bypass,
                replica_groups=[list(range(NC))],
                ins=[tbl_sh.opt()], outs=[table.opt()])

            # ---- main loop
            rep_ctx = tc.For_i(0, main_repeat, 1) if main_repeat > 1 else None
            if rep_ctx is not None:
                rep_ctx.__enter__()
            for w in range(nW):
                sidx = sp.tile([128, K], i32, tag="si")
                nc.sync.dma_start(out=sidx[:], in_=sidx_e[w])
                dloc = sp.tile([128, K], f32, tag="dl")
                nc.sync.dma_start(out=dloc[:], in_=dloc_e[w])
                nid = sp.tile([128, 1], i32, tag="ni")
                nc.sync.dma_start(out=nid[:], in_=nid_e[w])
                nrows = sp.tile([128, ROW], f32, tag="nr")
                nc.gpsimd.indirect_dma_start(
                    out=nrows[:], out_offset=None, in_=table[:],
                    in_offset=bass.IndirectOffsetOnAxis(ap=nid[:], axis=0))
                # qb[p, n] = q_dst of window node n (broadcast to all p)
                qT_ps = pp2.tile([1, 128], f32, space="PSUM", tag="qt")
                nc.tensor.transpose(out=qT_ps[:], in_=nrows[:, 131:132],
                                    identity=ident[:])
                qrow = sp.tile([1, 128], f32, tag="qr")
                nc.vector.tensor_copy(qrow[:], qT_ps[:])
                qb_ps = pp2.tile([128, 128], f32, space="PSUM", tag="qb")
                nc.tensor.matmul(out=qb_ps[:], lhsT=ones_row[:], rhs=qrow[:],
                                 start=True, stop=True)
                qb = sp.tile([128, 128], f32, tag="qbs")
                nc.vector.tensor_copy(qb[:], qb_ps[:])

                acc = pp2.tile([128, 65], f32, space="PSUM", tag="acc")
                for k in range(K):
                    pay = sp.tile([128, ROW], f32, tag="pay", bufs=6)
                    nc.gpsimd.indirect_dma_start(
                        out=pay[:], out_offset=None, in_=table[:],
                        in_offset=bass.IndirectOffsetOnAxis(
                            ap=sidx[:, k:k + 1], axis=0))
                    S01 = sp.tile([128, 128], f32, tag="s01", bufs=4)
                    nc.vector.tensor_scalar(
                        out=S01[:], in0=iotaf[:], scalar1=dloc[:, k:k + 1],
                        scalar2=None, op0=mybir.AluOpType.is_equal)
                    scr = sp.tile([128, 128], f32, tag="scr", bufs=2)
                    nc.vector.tensor_tensor(scr[:], S01[:], qb[:],
                                            op=mybir.AluOpType.mult)
                    qd = sp.tile([128, 1], f32, tag="qd", bufs=4)
                    nc.vector.tensor_reduce(
                        out=qd[:], in_=scr[:], axis=mybir.AxisListType.X,
                        op=mybir.AluOpType.add)
                    th = sp.tile([128, 1], f32, tag="th", bufs=4)
                    nc.scalar.activation(
                        out=th[:], in_=pay[:, 65:66],
                        func=mybir.ActivationFunctionType.Tanh,
                        bias=qd[:], scale=-1.0)
                    wc = sp.tile([128, 1], f32, tag="wc", bufs=4)
                    nc.scalar.activation(
                        out=wc[:], in_=th[:],
                        func=mybir.ActivationFunctionType.Exp)
                    sc = sp.tile([128, 65], f32, tag="sc", bufs=4)
                    nc.scalar.activation(
                        out=sc[:], in_=pay[:, 0:65],
                        func=mybir.ActivationFunctionType.Copy,
                        scale=wc[:])
                    nc.tensor.matmul(out=acc[:], lhsT=S01[:], rhs=sc[:],
                                     start=(k == 0), stop=(k == K - 1))

                # epilogue: out = elu(p_dst - swp/sw) * (sw != 0)
                z = sp.tile([128, 1], f32, tag="z")
                nc.vector.tensor_scalar(
                    out=z[:], in0=acc[:, 64:65], scalar1=0.0, scalar2=None,
                    op0=mybir.AluOpType.is_equal)
                den = sp.tile([128, 1], f32, tag="den")
                nc.vector.tensor_tensor(den[:], acc[:, 64:65], z[:],
                                        op=mybir.AluOpType.add)
                rec = sp.tile([128, 1], f32, tag="rec")
                nc.vector.reciprocal(rec[:], den[:])
                nzm = sp.tile([128, 1], f32, tag="nzm")
                nc.vector.scalar_tensor_tensor(
                    out=nzm[:], in0=z[:], scalar=-1.0, in1=ones_col[:],
                    op0=mybir.AluOpType.mult, op1=mybir.AluOpType.add)
                mean = sp.tile([128, D], f32, tag="mean")
                nc.vector.tensor_scalar(
                    out=mean[:], in0=acc[:, 0:64], scalar1=rec[:],
                    scalar2=None, op0=mybir.AluOpType.mult)
                diff = sp.tile([128, D], f32, tag="diff")
                nc.vector.tensor_tensor(diff[:], nrows[:, 66:130], mean[:],
                                        op=mybir.AluOpType.subtract)
                dm = sp.tile([128, D], f32, tag="dm")
                nc.vector.tensor_scalar(
                    out=dm[:], in0=diff[:], scalar1=nzm[:], scalar2=None,
                    op0=mybir.AluOpType.mult)
                neg = sp.tile([128, D], f32, tag="neg")
                nc.vector.tensor_scalar(
                    out=neg[:], in0=dm[:], scalar1=0.0, scalar2=None,
                    op0=mybir.AluOpType.min)
                ex = sp.tile([128, D], f32, tag="ex")
                nc.scalar.activation(out=ex[:], in_=neg[:],
                                     func=mybir.ActivationFunctionType.Exp)
                pos = sp.tile([128, D], f32, tag="pos")
                nc.vector.tensor_scalar(
                    out=pos[:], in0=dm[:], scalar1=0.0, scalar2=None,
                    op0=mybir.AluOpType.max)
                res = sp.tile([128, D], f32, tag="res")
                nc.vector.scalar_tensor_tensor(
                    out=res[:], in0=ex[:], scalar=-1.0, in1=pos[:],
                    op0=mybir.AluOpType.add, op1=mybir.AluOpType.add)
                nc.sync.dma_start(out=res_e[w * 128:(w + 1) * 128, :],
                                  in_=res[:])
            if rep_ctx is not None:
                rep_ctx.__exit__(None, None, None)
            pp2.release()
    nc.compile()
    return nc


_CACHE = {}


def _get_program(nW, main_repeat):
    key = (nW, main_repeat)
    if key not in _CACHE:
        _CACHE[key] = _build_program(nW, main_repeat)
    return _CACHE[key]


def kernel(h_src, h_dst, W_fc, w_attn, src, dst, _main_repeat=MAIN_REPEAT,
           _return_walls=False):
    from concourse.bass_utils import run_bass_kernel_spmd

    h_src = np.ascontiguousarray(np.asarray(h_src, np.float32))
    h_dst = np.ascontiguousarray(np.asarray(h_dst, np.float32))
    W_fc = np.ascontiguousarray(np.asarray(W_fc, np.float32))
    w_attn = np.ascontiguousarray(np.asarray(w_attn, np.float32)).reshape(D, 1)
    cores, nW = _prep(src, dst)

    hp = np.zeros((NPAD, D), np.float32); hp[:N] = h_src
    hq = np.zeros((NPAD, D), np.float32); hq[:N] = h_dst

    in_maps = []
    for c, core in enumerate(cores):
        in_maps.append({
            "hs": hp[c * SHARD:(c + 1) * SHARD],
            "hd": hq[c * SHARD:(c + 1) * SHARD],
            "wfc": W_fc,
            "wat": w_attn,
            "sidx": _to_tiles(core["src"]),
            "dloc": _to_tiles(core["dst_local"]),
            "nid": core["slot_ids"][:, :, None],
            })
    nc = _get_program(nW, _main_repeat)
    import time
    walls = []
    t0 = time.time()
    res = run_bass_kernel_spmd(nc, in_maps, list(range(NC)))
    walls.append(time.time() - t0)

    out = np.zeros((N, D), np.float32)
    for c, core in enumerate(cores):
        r = res.results[c]["res"].reshape(nW, 128, D)
        base, nn = core["base"], core["nn"]
        for w in range(nW):
            if nn[w] > 0:
                out[base[w]:base[w] + nn[w]] = r[w, :nn[w]]
    if _return_walls:
        return out, walls
    return out


if __name__ == "__main__":
    d = np.load("/root/problem/refdata.npz")
    out = kernel(d["h_src"], d["h_dst"], d["W_fc"], d["w_attn"],
                 d["src"], d["dst"])
    exp = d["expected"]
    rel = np.linalg.norm(out - exp) / np.linalg.norm(exp)
    print(f"rel_l2 = {rel:.3e}  maxabs = {np.abs(out - exp).max():.3e}")
